# Initial kernel scaffold
#
"""Causal single-head attention (B=8, N=2048, D=H=1024, fp32) on 8 TRN2 cores.

Data-parallel: one batch element per NeuronCore. Host pre-transposes x so
activations live in [D, N] ("transposed") layout on chip; scores are computed
transposed (S^T = K @ Q^T, [key, query]) so the attention @ V matmul needs no
on-chip transposes. Softmax runs over the partition (key) axis: exp on the
scalar engine, causal mask via affine_select on diagonal tiles, row-sums via
N=1 matmuls against a ones vector, and the normalization division is folded
into the output eviction as a per-partition scale.

Matmuls run in float32r (single-pass fp32, ~1.5e-4 rel err) by default;
set ATTN_MM_DTYPE=float32 for exact-fp32 (4x slower) matmuls.
"""

import os
import sys
from contextlib import ExitStack

import numpy as np

# The concourse/bass toolchain comes from the container's python path; fall
# back to the /opt copy when running outside the preconfigured interpreter.
try:
    import concourse.bacc as bacc
except ImportError:  # pragma: no cover
    sys.path.insert(0, "/opt/trn_rl_repo")
    import concourse.bacc as bacc

import concourse.mybir as mybir
from concourse.tile import TileContext
from concourse.bass_utils import run_bass_kernel_spmd

# bass_utils imports antenv.axon_hooks when BASS_TRACE is set; provide a stub
# so tracing degrades gracefully instead of crashing if the module is absent.
try:
    import antenv.axon_hooks  # noqa: F401
except ImportError:  # pragma: no cover
    import types

    _m = types.ModuleType("antenv.axon_hooks")
    _m._hook = None
    _m.set_axon_ntff_profile_hook = lambda h: setattr(_m, "_hook", h)
    _m.get_axon_ntff_profile_hook = lambda: _m._hook
    sys.modules["antenv.axon_hooks"] = _m

B, N, D, H = 8, 2048, 1024, 1024
P = 128
DT = D // P          # 8 contraction tiles for the projections
HT = H // P          # 8 h-tiles
NT = N // P          # 16 sequence tiles of 128
IT = N // 512        # 4 query tiles of 512
SCALE = 1.0 / np.sqrt(float(H))

F32 = mybir.dt.float32

LAST_RESULT = None  # BassKernelResults of the most recent kernel() call
_CACHE = {}


def build_program(mm_dtype_name: str):
    mm_dt = getattr(mybir.dt, mm_dtype_name)
    nc = bacc.Bacc("TRN2", target_bir_lowering=False, debug=False)

    xT = nc.dram_tensor("xT", [4, D, 512], mm_dt, kind="ExternalInput")
    Wq = nc.dram_tensor("Wq", [2, D, 512], mm_dt, kind="ExternalInput")
    Wk = nc.dram_tensor("Wk", [2, D, 512], mm_dt, kind="ExternalInput")
    Wv = nc.dram_tensor("Wv", [2, D, 512], mm_dt, kind="ExternalInput")
    bqT = nc.dram_tensor("bqT", [P, HT], F32, kind="ExternalInput")
    bkT = nc.dram_tensor("bkT", [P, HT], F32, kind="ExternalInput")
    bvB = nc.dram_tensor("bvB", [P, H], F32, kind="ExternalInput")
    out = nc.dram_tensor("out", [N, H], F32, kind="ExternalOutput")

    Exp = mybir.ActivationFunctionType.Exp
    Identity = mybir.ActivationFunctionType.Identity
    Copy = mybir.ActivationFunctionType.Copy

    with TileContext(nc) as tc:
        with ExitStack() as top:
            const = top.enter_context(tc.tile_pool(name="const", bufs=1))
            kt_pool = top.enter_context(tc.tile_pool(name="kt", bufs=1))
            # qt + the score/rowsum PSUM pools live at the bottom of the
            # allocation stack so phase 2's score chain doesn't pick up
            # address-reuse dependencies on late phase-1 work.
            qt_pool = top.enter_context(tc.tile_pool(name="qt", bufs=2))
            ps_s = top.enter_context(tc.tile_pool(name="pss", bufs=2, space="PSUM"))
            ps_rs = top.enter_context(tc.tile_pool(name="psrs", bufs=1, space="PSUM"))
            dram = top.enter_context(tc.tile_pool(name="dram", bufs=1, space="DRAM"))

            ones = const.tile([P, 1], F32, tag="ones")
            nc.vector.memset(ones[:], 1.0)
            bq_sb = const.tile([P, HT], F32, tag="bq")
            bk_sb = const.tile([P, HT], F32, tag="bk")


            qt_dram = dram.tile([H, N], mm_dt, tag="qt_dram")
            v_dram = dram.tile([N, H], mm_dt, tag="v_dram")

            kt = [kt_pool.tile([P, N], mm_dt, tag=f"kt{h}", name=f"kt{h}") for h in range(HT)]

            # ---------------- Phase 1: projections (Q, K, then V) ----------------
            with ExitStack() as p1:
                xt_pool = p1.enter_context(tc.tile_pool(name="xt", bufs=1))
                w_pool = p1.enter_context(tc.tile_pool(name="w", bufs=1))
                stg = p1.enter_context(tc.tile_pool(name="stg", bufs=2))
                ps1 = p1.enter_context(tc.tile_pool(name="ps1", bufs=5, space="PSUM"))

                bv_sb = stg.tile([P, H], F32, tag="bv", bufs=1)
                nc.sync.dma_start(bv_sb[:], bvB.ap()[:, :])

                xt = [xt_pool.tile([P, N], mm_dt, tag=f"xt{d}", name=f"xt{d}") for d in range(DT)]

                # Weight tiles are split into column halves with separate pool
                # tags: a projection consumes its A-half first, so the next
                # projection's A-half loads overlap this one's B-half compute.
                def alloc_w():
                    wa = [w_pool.tile([P, 512], mm_dt, tag=f"wa{d}", name=f"wa{d}") for d in range(DT)]
                    wb = [w_pool.tile([P, 512], mm_dt, tag=f"wb{d}", name=f"wb{d}") for d in range(DT)]
                    return wa, wb

                def load_w_half(tiles, w_dram, half):
                    for d in range(DT):
                        nc.sync.dma_start(
                            tiles[d][:],
                            w_dram.ap()[half, d * P:(d + 1) * P, :],
                        )

                def w_lhsT(wa, wb, d, h):
                    half = wa if h < HT // 2 else wb
                    hh = h % (HT // 2)
                    return half[d][:, hh * P:(hh + 1) * P]

                def proj_groups(wa, wb, evict):
                    """Emit psum groups half-by-half to match DMA wave order."""
                    for hh in range(2):
                        for h in range(hh * 4, hh * 4 + 4):
                            for nch in range(4):
                                ps = ps1.tile([P, 512], F32, tag="ps")
                                for d in range(DT):
                                    nc.tensor.matmul(
                                        ps[:],
                                        w_lhsT(wa, wb, d, h),
                                        xt[d][:, nch * 512:(nch + 1) * 512],
                                        start=(d == 0),
                                        stop=(d == DT - 1),
                                    )
                                evict(ps, h, nch)

                # --- Q^T = Wq^T @ x^T (+ bq), spilled to DRAM ---
                # DMA wave order: Wq A-half, x^T columns, Wq B-half.
                wa, wb = alloc_w()
                def load_xt_col(c):
                    for d in range(DT):
                        nc.sync.dma_start(
                            xt[d][:, c * 512:(c + 1) * 512],
                            xT.ap()[c, d * P:(d + 1) * P, :],
                        )

                # First (d=0) pair split across partition quarters so all 8
                # DMA queues deliver it concurrently — it gates the very
                # first matmul.
                for q in range(4):
                    nc.sync.dma_start(
                        wa[0][q * 32:(q + 1) * 32, :],
                        Wq.ap()[0, q * 32:(q + 1) * 32, :],
                    )
                    nc.sync.dma_start(
                        xt[0][q * 32:(q + 1) * 32, 0:512],
                        xT.ap()[0, q * 32:(q + 1) * 32, :],
                    )
                for d in range(1, DT):
                    nc.sync.dma_start(wa[d][:], Wq.ap()[0, d * P:(d + 1) * P, :])
                    nc.sync.dma_start(
                        xt[d][:, 0:512], xT.ap()[0, d * P:(d + 1) * P, :]
                    )
                nc.sync.dma_start(bq_sb[:], bqT.ap()[:, :])
                nc.sync.dma_start(bk_sb[:], bkT.ap()[:, :])
                load_xt_col(1)
                load_w_half(wb, Wq, 1)
                load_xt_col(2)
                load_xt_col(3)

                def evict_q(ps, h, nch):
                    st = stg.tile([P, 512], mm_dt, tag="st", bufs=3, name="st")
                    if (h + nch) % 2 == 0:
                        nc.scalar.activation(
                            st[:], ps[:], Identity, bias=bq_sb[:, h:h + 1]
                        )
                    else:
                        nc.vector.tensor_scalar_add(st[:], ps[:], bq_sb[:, h:h + 1])
                    nc.sync.dma_start(
                        qt_dram[h * P:(h + 1) * P, nch * 512:(nch + 1) * 512], st[:]
                    )

                proj_groups(wa, wb, evict_q)

                # --- K^T = Wk^T @ x^T (+ bk), kept resident ---
                wa, wb = alloc_w()
                load_w_half(wa, Wk, 0)
                load_w_half(wb, Wk, 1)

                def evict_k(ps, h, nch):
                    dst = kt[h][:, nch * 512:(nch + 1) * 512]
                    if (h + nch) % 2 == 0:
                        nc.scalar.activation(dst, ps[:], Identity, bias=bk_sb[:, h:h + 1])
                    else:
                        nc.vector.tensor_scalar_add(dst, ps[:], bk_sb[:, h:h + 1])

                proj_groups(wa, wb, evict_k)

                # --- V = x @ Wv (+ bv), spilled to DRAM ---
                # Last so its PE work overlaps the first phase-2 score tiles.
                wa, wb = alloc_w()
                load_w_half(wa, Wv, 0)
                load_w_half(wb, Wv, 1)
                for n in range(NT):
                    for hch in range(2):
                        whalf = wa if hch == 0 else wb
                        ps = ps1.tile([P, 512], F32, tag="ps")
                        for d in range(DT):
                            nc.tensor.matmul(
                                ps[:],
                                xt[d][:, n * P:(n + 1) * P],
                                whalf[d][:],
                                start=(d == 0),
                                stop=(d == DT - 1),
                            )
                        vst = stg.tile([P, 512], mm_dt, tag="vst", bufs=2, name="vst")
                        nc.vector.tensor_add(
                            vst[:],
                            ps[:],
                            bv_sb[:, hch * 512:(hch + 1) * 512],
                        )
                        nc.sync.dma_start(
                            v_dram[n * P:(n + 1) * P, hch * 512:(hch + 1) * 512],
                            vst[:],
                        )

            # ---------------- Phase 2: attention ----------------
            with ExitStack() as p2:
                v_pool = p2.enter_context(tc.tile_pool(name="vp", bufs=1))
                pt_pool = p2.enter_context(tc.tile_pool(name="pt", bufs=1))
                out_pool = p2.enter_context(tc.tile_pool(name="op", bufs=2))
                sm_pool = p2.enter_context(tc.tile_pool(name="sm", bufs=4))
                ps_av = p2.enter_context(tc.tile_pool(name="psav", bufs=4, space="PSUM"))

                vt_cache = []
                for t in range(IT):
                    i0 = 512 * t
                    jmax = 4 * t + 3

                    qt = [qt_pool.tile([P, 512], mm_dt, tag=f"qt{h}", name=f"qt{h}") for h in range(HT)]
                    for h in range(HT):
                        nc.sync.dma_start(
                            qt[h][:], qt_dram[h * P:(h + 1) * P, i0:i0 + 512]
                        )
                    # V j-tiles are loaded once at first use and stay resident
                    # (their per-j pool slots are reserved for the whole phase
                    # anyway, so re-streaming them per i-tile was pure waste).
                    for j in range(len(vt_cache), jmax + 1):
                        v = v_pool.tile([P, H], mm_dt, tag=f"v{j}", name=f"v{j}")
                        nc.sync.dma_start(v[:], v_dram[j * P:(j + 1) * P, :])
                        vt_cache.append(v)
                    vt = vt_cache

                    # scores^T [key j, query i] + exp + causal mask.
                    # Diagonal j-tiles only compute the causally-live column
                    # range [c, 512); columns below c are never read (the AV
                    # lhsT slice for i-sub s starts at 128*s >= c).
                    pt = []
                    for j in range(jmax + 1):
                        c = max(0, j * P - i0)
                        w_ = 512 - c
                        ps = ps_s.tile([P, 512], F32, tag="ps")
                        for h in range(HT):
                            nc.tensor.matmul(
                                ps[:, 0:w_],
                                kt[h][:, j * P:(j + 1) * P],
                                qt[h][:, c:512],
                                start=(h == 0),
                                stop=(h == HT - 1),
                            )
                        p = pt_pool.tile([P, 512], mm_dt, tag=f"pt{j}", name=f"pt{j}")
                        nc.scalar.activation(
                            p[:, c:512], ps[:, 0:w_], Exp, scale=float(SCALE)
                        )
                        if c > 0 or j * P == i0:
                            # keep exp where key j*P+p <= query i0+c+f', else 0
                            nc.gpsimd.affine_select(
                                out=p[:, c:512],
                                in_=p[:, c:512],
                                compare_op=mybir.AluOpType.is_ge,
                                fill=0.0,
                                base=0,
                                channel_multiplier=-1,
                                pattern=[[1, w_]],
                            )
                        pt.append(p)

                    # attn @ V, row-sums, normalize on eviction
                    for s in range(4):
                        g = 4 * t + s
                        pav = [ps_av.tile([P, 512], F32, tag="pav", name="pav") for _ in range(2)]
                        prs = ps_rs.tile([P, 1], F32, tag="prs")
                        for j in range(g + 1):
                            lhsT = pt[j][:, s * P:(s + 1) * P]
                            for hch in range(2):
                                nc.tensor.matmul(
                                    pav[hch][:],
                                    lhsT,
                                    vt[j][:, hch * 512:(hch + 1) * 512],
                                    start=(j == 0),
                                    stop=(j == g),
                                )
                            nc.tensor.matmul(
                                prs[:],
                                lhsT.bitcast(F32),
                                ones[:],
                                start=(j == 0),
                                stop=(j == g),
                            )
                        recip = sm_pool.tile([P, 1], F32, tag="recip")
                        nc.vector.reciprocal(recip[:], prs[:])
                        ot = out_pool.tile([P, H], F32, tag="ot")
                        for hch in range(2):
                            nc.scalar.activation(
                                ot[:, hch * 512:(hch + 1) * 512],
                                pav[hch][:],
                                Copy,
                                scale=recip[:],
                            )
                        nc.sync.dma_start(
                            out.ap()[i0 + s * P:i0 + (s + 1) * P, :], ot[:]
                        )

    nc.compile()
    return nc


def _get_program():
    name = os.environ.get("ATTN_MM_DTYPE", "float32r")
    if name not in _CACHE:
        _CACHE[name] = build_program(name)
    return _CACHE[name]


def kernel(x, Wq, bq, Wk, bk, Wv, bv):
    global LAST_RESULT
    x = np.asarray(x, dtype=np.float32)
    Wq = np.asarray(Wq, dtype=np.float32)
    Wk = np.asarray(Wk, dtype=np.float32)
    Wv = np.asarray(Wv, dtype=np.float32)
    bq = np.asarray(bq, dtype=np.float32)
    bk = np.asarray(bk, dtype=np.float32)
    bv = np.asarray(bv, dtype=np.float32)

    nc = _get_program()

    def col_chunks(m, n_chunks):
        # [D, n_chunks*512] -> [n_chunks, D, 512], each chunk contiguous
        return np.ascontiguousarray(
            m.reshape(D, n_chunks, 512).transpose(1, 0, 2)
        )

    xT_b = [col_chunks(np.ascontiguousarray(x[b].T), 4) for b in range(B)]
    Wq_c = col_chunks(Wq, 2)
    Wk_c = col_chunks(Wk, 2)
    Wv_c = col_chunks(Wv, 2)

    bqT = np.ascontiguousarray(bq.reshape(HT, P).T)
    bkT = np.ascontiguousarray(bk.reshape(HT, P).T)
    bvB = np.ascontiguousarray(np.broadcast_to(bv, (P, H)))

    in_maps = []
    for b in range(B):
        in_maps.append(
            {
                "xT": xT_b[b],
                "Wq": Wq_c,
                "Wk": Wk_c,
                "Wv": Wv_c,
                "bqT": bqT,
                "bkT": bkT,
                "bvB": bvB,
            }
        )

    res = run_bass_kernel_spmd(nc, in_maps, core_ids=list(range(B)))
    LAST_RESULT = res
    return np.stack([res.results[b]["out"] for b in range(B)], axis=0)



# revision 1
# speedup vs baseline: 1.3610x; 1.3610x over previous
"""Causal single-head attention (B=8, N=2048, D=H=1024, fp32) on 8 TRN2 cores.

Data-parallel: one batch element per NeuronCore. Host pre-transposes x so
activations live in [D, N] ("transposed") layout on chip; scores are computed
transposed (S^T = K @ Q^T, [key, query]) so the attention @ V matmul needs no
on-chip transposes. Softmax runs over the partition (key) axis: exp on the
scalar engine, causal mask via affine_select on diagonal tiles, row-sums via
N=1 matmuls against a ones vector, and the normalization division is folded
into the output eviction as a per-partition scale.

Matmuls run in float32r (single-pass fp32, ~1.5e-4 rel err) by default;
set ATTN_MM_DTYPE=float32 for exact-fp32 (4x slower) matmuls.
"""

import os
import sys
from contextlib import ExitStack

import numpy as np

# The concourse/bass toolchain comes from the container's python path; fall
# back to the /opt copy when running outside the preconfigured interpreter.
try:
    import concourse.bacc as bacc
except ImportError:  # pragma: no cover
    sys.path.insert(0, "/opt/trn_rl_repo")
    import concourse.bacc as bacc

import concourse.mybir as mybir
from concourse.tile import TileContext
from concourse.bass_utils import run_bass_kernel_spmd

# bass_utils imports antenv.axon_hooks when BASS_TRACE is set; provide a stub
# so tracing degrades gracefully instead of crashing if the module is absent.
try:
    import antenv.axon_hooks  # noqa: F401
except ImportError:  # pragma: no cover
    import types

    _m = types.ModuleType("antenv.axon_hooks")
    _m._hook = None
    _m.set_axon_ntff_profile_hook = lambda h: setattr(_m, "_hook", h)
    _m.get_axon_ntff_profile_hook = lambda: _m._hook
    sys.modules["antenv.axon_hooks"] = _m

B, N, D, H = 8, 2048, 1024, 1024
P = 128
DT = D // P          # 8 contraction tiles for the projections
HT = H // P          # 8 h-tiles
NT = N // P          # 16 sequence tiles of 128
IT = N // 512        # 4 query tiles of 512
SCALE = 1.0 / np.sqrt(float(H))

F32 = mybir.dt.float32

LAST_RESULT = None  # BassKernelResults of the most recent kernel() call
_CACHE = {}


def build_program(mm_dtype_name: str):
    mm_dt = getattr(mybir.dt, mm_dtype_name)
    nc = bacc.Bacc("TRN2", target_bir_lowering=False, debug=False)

    xT = nc.dram_tensor("xT", [4, D, 512], mm_dt, kind="ExternalInput")
    Wq = nc.dram_tensor("Wq", [2, D, 512], mm_dt, kind="ExternalInput")
    Wk = nc.dram_tensor("Wk", [2, D, 512], mm_dt, kind="ExternalInput")
    Wv = nc.dram_tensor("Wv", [2, D, 512], mm_dt, kind="ExternalInput")
    bqT = nc.dram_tensor("bqT", [P, HT], F32, kind="ExternalInput")
    bkT = nc.dram_tensor("bkT", [P, HT], F32, kind="ExternalInput")
    bvB = nc.dram_tensor("bvB", [P, H], F32, kind="ExternalInput")
    out = nc.dram_tensor("out", [N, H], F32, kind="ExternalOutput")

    Exp = mybir.ActivationFunctionType.Exp
    Identity = mybir.ActivationFunctionType.Identity
    Copy = mybir.ActivationFunctionType.Copy

    with TileContext(nc) as tc:
        with ExitStack() as top:
            const = top.enter_context(tc.tile_pool(name="const", bufs=1))
            kt_pool = top.enter_context(tc.tile_pool(name="kt", bufs=1))
            # qt + the score/rowsum PSUM pools live at the bottom of the
            # allocation stack so phase 2's score chain doesn't pick up
            # address-reuse dependencies on late phase-1 work.
            qt_pool = top.enter_context(tc.tile_pool(name="qt", bufs=2))
            ps_s = top.enter_context(tc.tile_pool(name="pss", bufs=2, space="PSUM"))
            ps_rs = top.enter_context(tc.tile_pool(name="psrs", bufs=1, space="PSUM"))
            dram = top.enter_context(tc.tile_pool(name="dram", bufs=1, space="DRAM"))

            ones = const.tile([P, 1], F32, tag="ones")
            nc.vector.memset(ones[:], 1.0)
            bq_sb = const.tile([P, HT], F32, tag="bq")
            bk_sb = const.tile([P, HT], F32, tag="bk")


            qt_dram = dram.tile([H, N], mm_dt, tag="qt_dram")
            v_dram = dram.tile([N, H], mm_dt, tag="v_dram")

            kt = [kt_pool.tile([P, N], mm_dt, tag=f"kt{h}", name=f"kt{h}") for h in range(HT)]

            # ---------------- Phase 1: projections (Q, K, then V) ----------------
            with ExitStack() as p1:
                xt_pool = p1.enter_context(tc.tile_pool(name="xt", bufs=1))
                w_pool = p1.enter_context(tc.tile_pool(name="w", bufs=1))
                stg = p1.enter_context(tc.tile_pool(name="stg", bufs=2))
                ps1 = p1.enter_context(tc.tile_pool(name="ps1", bufs=5, space="PSUM"))

                bv_sb = stg.tile([P, H], F32, tag="bv", bufs=1)
                nc.sync.dma_start(bv_sb[:], bvB.ap()[:, :])

                xt = [xt_pool.tile([P, N], mm_dt, tag=f"xt{d}", name=f"xt{d}") for d in range(DT)]

                # Weight tiles are split into column halves with separate pool
                # tags: a projection consumes its A-half first, so the next
                # projection's A-half loads overlap this one's B-half compute.
                def alloc_w():
                    wa = [w_pool.tile([P, 512], mm_dt, tag=f"wa{d}", name=f"wa{d}") for d in range(DT)]
                    wb = [w_pool.tile([P, 512], mm_dt, tag=f"wb{d}", name=f"wb{d}") for d in range(DT)]
                    return wa, wb

                def load_w_half(tiles, w_dram, half):
                    for d in range(DT):
                        nc.sync.dma_start(
                            tiles[d][:],
                            w_dram.ap()[half, d * P:(d + 1) * P, :],
                        )

                def w_lhsT(wa, wb, d, h):
                    half = wa if h < HT // 2 else wb
                    hh = h % (HT // 2)
                    return half[d][:, hh * P:(hh + 1) * P]

                def proj_groups(wa, wb, evict):
                    """Emit psum groups half-by-half to match DMA wave order."""
                    for hh in range(2):
                        for h in range(hh * 4, hh * 4 + 4):
                            for nch in range(4):
                                ps = ps1.tile([P, 512], F32, tag="ps")
                                for d in range(DT):
                                    nc.tensor.matmul(
                                        ps[:],
                                        w_lhsT(wa, wb, d, h),
                                        xt[d][:, nch * 512:(nch + 1) * 512],
                                        start=(d == 0),
                                        stop=(d == DT - 1),
                                    )
                                evict(ps, h, nch)

                # --- Q^T = Wq^T @ x^T (+ bq), spilled to DRAM ---
                # DMA wave order: Wq A-half, x^T columns, Wq B-half.
                wa, wb = alloc_w()
                def load_xt_col(c):
                    for d in range(DT):
                        nc.sync.dma_start(
                            xt[d][:, c * 512:(c + 1) * 512],
                            xT.ap()[c, d * P:(d + 1) * P, :],
                        )

                # First (d=0) pair split across partition quarters so all 8
                # DMA queues deliver it concurrently — it gates the very
                # first matmul.
                for q in range(4):
                    nc.sync.dma_start(
                        wa[0][q * 32:(q + 1) * 32, :],
                        Wq.ap()[0, q * 32:(q + 1) * 32, :],
                    )
                    nc.sync.dma_start(
                        xt[0][q * 32:(q + 1) * 32, 0:512],
                        xT.ap()[0, q * 32:(q + 1) * 32, :],
                    )
                for d in range(1, DT):
                    nc.sync.dma_start(wa[d][:], Wq.ap()[0, d * P:(d + 1) * P, :])
                    nc.sync.dma_start(
                        xt[d][:, 0:512], xT.ap()[0, d * P:(d + 1) * P, :]
                    )
                nc.sync.dma_start(bq_sb[:], bqT.ap()[:, :])
                nc.sync.dma_start(bk_sb[:], bkT.ap()[:, :])
                load_xt_col(1)
                load_w_half(wb, Wq, 1)
                load_xt_col(2)
                load_xt_col(3)

                def evict_q(ps, h, nch):
                    st = stg.tile([P, 512], mm_dt, tag="st", bufs=3, name="st")
                    if (h + nch) % 2 == 0:
                        nc.scalar.activation(
                            st[:], ps[:], Identity, bias=bq_sb[:, h:h + 1]
                        )
                    else:
                        nc.vector.tensor_scalar_add(st[:], ps[:], bq_sb[:, h:h + 1])
                    nc.sync.dma_start(
                        qt_dram[h * P:(h + 1) * P, nch * 512:(nch + 1) * 512], st[:]
                    )

                proj_groups(wa, wb, evict_q)

                # --- K^T = Wk^T @ x^T (+ bk), kept resident ---
                wa, wb = alloc_w()
                load_w_half(wa, Wk, 0)
                load_w_half(wb, Wk, 1)

                def evict_k(ps, h, nch):
                    dst = kt[h][:, nch * 512:(nch + 1) * 512]
                    if (h + nch) % 2 == 0:
                        nc.scalar.activation(dst, ps[:], Identity, bias=bk_sb[:, h:h + 1])
                    else:
                        nc.vector.tensor_scalar_add(dst, ps[:], bk_sb[:, h:h + 1])

                proj_groups(wa, wb, evict_k)

                # --- V = x @ Wv (+ bv), spilled to DRAM ---
                # Last so its PE work overlaps the first phase-2 score tiles.
                wa, wb = alloc_w()
                load_w_half(wa, Wv, 0)
                load_w_half(wb, Wv, 1)
                for n in range(NT):
                    for hch in range(2):
                        whalf = wa if hch == 0 else wb
                        ps = ps1.tile([P, 512], F32, tag="ps")
                        for d in range(DT):
                            nc.tensor.matmul(
                                ps[:],
                                xt[d][:, n * P:(n + 1) * P],
                                whalf[d][:],
                                start=(d == 0),
                                stop=(d == DT - 1),
                            )
                        vst = stg.tile([P, 512], mm_dt, tag="vst", bufs=2, name="vst")
                        nc.vector.tensor_add(
                            vst[:],
                            ps[:],
                            bv_sb[:, hch * 512:(hch + 1) * 512],
                        )
                        nc.sync.dma_start(
                            v_dram[n * P:(n + 1) * P, hch * 512:(hch + 1) * 512],
                            vst[:],
                        )

            # ---------------- Phase 2: attention ----------------
            with ExitStack() as p2:
                v_pool = p2.enter_context(tc.tile_pool(name="vp", bufs=1))
                pt_pool = p2.enter_context(tc.tile_pool(name="pt", bufs=1))
                out_pool = p2.enter_context(tc.tile_pool(name="op", bufs=2))
                sm_pool = p2.enter_context(tc.tile_pool(name="sm", bufs=4))
                ps_av = p2.enter_context(tc.tile_pool(name="psav", bufs=4, space="PSUM"))

                vt_cache = []
                for t in range(IT):
                    i0 = 512 * t
                    jmax = 4 * t + 3

                    qt = [qt_pool.tile([P, 512], mm_dt, tag=f"qt{h}", name=f"qt{h}") for h in range(HT)]
                    for h in range(HT):
                        nc.sync.dma_start(
                            qt[h][:], qt_dram[h * P:(h + 1) * P, i0:i0 + 512]
                        )
                    # V j-tiles are loaded once at first use and stay resident
                    # (their per-j pool slots are reserved for the whole phase
                    # anyway, so re-streaming them per i-tile was pure waste).
                    for j in range(len(vt_cache), jmax + 1):
                        v = v_pool.tile([P, H], mm_dt, tag=f"v{j}", name=f"v{j}")
                        nc.sync.dma_start(v[:], v_dram[j * P:(j + 1) * P, :])
                        vt_cache.append(v)
                    vt = vt_cache

                    # scores^T [key j, query i] + exp + causal mask.
                    # Diagonal j-tiles only compute the causally-live column
                    # range [c, 512); columns below c are never read (the AV
                    # lhsT slice for i-sub s starts at 128*s >= c).
                    pt = []
                    for j in range(jmax + 1):
                        c = max(0, j * P - i0)
                        w_ = 512 - c
                        ps = ps_s.tile([P, 512], F32, tag="ps")
                        for h in range(HT):
                            nc.tensor.matmul(
                                ps[:, 0:w_],
                                kt[h][:, j * P:(j + 1) * P],
                                qt[h][:, c:512],
                                start=(h == 0),
                                stop=(h == HT - 1),
                            )
                        p = pt_pool.tile([P, 512], mm_dt, tag=f"pt{j}", name=f"pt{j}")
                        nc.scalar.activation(
                            p[:, c:512], ps[:, 0:w_], Exp, scale=float(SCALE)
                        )
                        if c > 0 or j * P == i0:
                            # keep exp where key j*P+p <= query i0+c+f', else 0
                            nc.gpsimd.affine_select(
                                out=p[:, c:512],
                                in_=p[:, c:512],
                                compare_op=mybir.AluOpType.is_ge,
                                fill=0.0,
                                base=0,
                                channel_multiplier=-1,
                                pattern=[[1, w_]],
                            )
                        pt.append(p)

                    # attn @ V, row-sums, normalize on eviction
                    for s in range(4):
                        g = 4 * t + s
                        pav = [ps_av.tile([P, 512], F32, tag="pav", name="pav") for _ in range(2)]
                        prs = ps_rs.tile([P, 1], F32, tag="prs")
                        for j in range(g + 1):
                            lhsT = pt[j][:, s * P:(s + 1) * P]
                            for hch in range(2):
                                nc.tensor.matmul(
                                    pav[hch][:],
                                    lhsT,
                                    vt[j][:, hch * 512:(hch + 1) * 512],
                                    start=(j == 0),
                                    stop=(j == g),
                                )
                            nc.tensor.matmul(
                                prs[:],
                                lhsT.bitcast(F32),
                                ones[:],
                                start=(j == 0),
                                stop=(j == g),
                            )
                        recip = sm_pool.tile([P, 1], F32, tag="recip")
                        nc.vector.reciprocal(recip[:], prs[:])
                        ot = out_pool.tile([P, H], F32, tag="ot")
                        for hch in range(2):
                            nc.scalar.activation(
                                ot[:, hch * 512:(hch + 1) * 512],
                                pav[hch][:],
                                Copy,
                                scale=recip[:],
                            )
                        nc.sync.dma_start(
                            out.ap()[i0 + s * P:i0 + (s + 1) * P, :], ot[:]
                        )

    nc.compile()
    return nc


def _get_program():
    name = os.environ.get("ATTN_MM_DTYPE", "float32r")
    if name not in _CACHE:
        _CACHE[name] = build_program(name)
    return _CACHE[name]


def kernel(x, Wq, bq, Wk, bk, Wv, bv):
    global LAST_RESULT
    x = np.asarray(x, dtype=np.float32)
    Wq = np.asarray(Wq, dtype=np.float32)
    Wk = np.asarray(Wk, dtype=np.float32)
    Wv = np.asarray(Wv, dtype=np.float32)
    bq = np.asarray(bq, dtype=np.float32)
    bk = np.asarray(bk, dtype=np.float32)
    bv = np.asarray(bv, dtype=np.float32)

    nc = _get_program()

    def col_chunks(m, n_chunks):
        # [D, n_chunks*512] -> [n_chunks, D, 512], each chunk contiguous
        return np.ascontiguousarray(
            m.reshape(D, n_chunks, 512).transpose(1, 0, 2)
        )

    xT_b = [col_chunks(np.ascontiguousarray(x[b].T), 4) for b in range(B)]
    Wq_c = col_chunks(Wq, 2)
    Wk_c = col_chunks(Wk, 2)
    Wv_c = col_chunks(Wv, 2)

    bqT = np.ascontiguousarray(bq.reshape(HT, P).T)
    bkT = np.ascontiguousarray(bk.reshape(HT, P).T)
    bvB = np.ascontiguousarray(np.broadcast_to(bv, (P, H)))

    in_maps = []
    for b in range(B):
        in_maps.append(
            {
                "xT": xT_b[b],
                "Wq": Wq_c,
                "Wk": Wk_c,
                "Wv": Wv_c,
                "bqT": bqT,
                "bkT": bkT,
                "bvB": bvB,
            }
        )

    res = run_bass_kernel_spmd(nc, in_maps, core_ids=list(range(B)))
    LAST_RESULT = res
    return np.stack([res.results[b]["out"] for b in range(B)], axis=0)

